# revision 29
# baseline (speedup 1.0000x reference)
"""Trainium2 Bass kernel for CARE position encoding (rotor sandwich product).

The reference computes out = R x R~ where R is a product of 4 plane rotors
(cos(phi_i) + sin(phi_i) e_mi) with phi_i = 0.5 * c_i * theta[pos, i].
Algebraically this factorizes into 4 sequential Givens-rotation stages: for
plane bivector e_m, the 8 basis blades A with |A & m| == 1 rotate in 4
disjoint pairs (A, A^m) by angle 2*phi with pair signs tau = C[A, m, A^m];
the other 8 blades pass through unchanged:
    out[a] = c2*x[a] + tau*s2*x[b] ;  out[b] = c2*x[b] - tau*s2*x[a]

Implementation (data-parallel across 8 cores, batch-sharded, 2 rows/core):
 - component-planar fp16 layout: each x tile is [128, 16 comp-blocks * JT]
   so every DVE tensor_tensor operand has a packed stride-1 innermost dim
   of JT fp16 elements -> the DVE runs them in 2x_1P mode (2 elem/cycle)
   instead of the 1x fp32 mode.
 - host marshals x into this layout (and fp16) per core; the device ladder
   runs four in-place Givens stages (mul cos, mul sin, add/sub split by the
   Cayley pair sign tau); output DMA'd back in fp16 and unpacked on host.
 - angles once per core: q_i = float(pos) * (f_i*c_i/2pi); round via the
   1.5*2^23 magic constant; FR = q - round(q) in [-1/2,1/2] so the rotation
   angle A_i == 2pi*FR (mod 2pi).  ScalarE Sin evaluates both tables with
   the affine pre-scale folded into the activation: s2 = Sin(2pi*FR),
   c2 = Sin(-2pi*|FR| + pi/2) = cos(A).
 - every stage's index arithmetic is verified symbolically against the
   input Cayley tensor at kernel() time.
"""
import numpy as np

import concourse.bass as bass
import concourse.tile as tile
from concourse import bacc, mybir
from concourse.bass_utils import run_bass_kernel_spmd

F32 = mybir.dt.float32
F16 = mybir.dt.float16
I16 = mybir.dt.int16
I32 = mybir.dt.int32
ALU = mybir.AluOpType

P = 128
NCORES = 8
B, L, MV = 16, 16384, 16
MAX_LEN = 16384
ROWS_PER_CORE = B // NCORES          # 2
N = ROWS_PER_CORE * L                # 32768 positions per core
J = N // P                           # 256 positions per partition
JT = J                               # single x-tile covering all positions
NT = J // JT

PLANE_BLADES = (3, 5, 9, 6)          # reference plane order (rotor build)
STAGE_ORDER = (6, 9, 5, 3)           # sandwich stage application order

MAGIC = float(np.float32(1.5 * 2 ** 23))
TWO_PI = float(2.0 * np.pi)
HALF_PI = float(np.pi / 2.0)

# Component permutation: device comp a' = SIGMA[a] for reference comp a.
# GF(2)-linear change of basis chosen so every stage's rotating set is a
# contiguous / single-stride block: b0'=bit0, b1'=bit0^bit3, b2'=bit0^bit2,
# b3'=bit0^bit1.  SIGMA is an involution (SIGMA == its inverse).  Comps 0/1
# (scalar + pseudoscalar) commute with every rotor and pass through
# untouched; the host copies them directly and the device skips them.
SIGMA = (0, 15, 8, 7, 4, 11, 12, 3, 2, 13, 10, 5, 6, 9, 14, 1)

# Per-stage op descriptors in permuted component-planar layout.  comp dims
# are in units of J columns; slot = rank of the destination comp a' within
# the sorted rotating set.  t/u read X at xoff+xdims, write T/U at
# toff+tdims; ap/am are the tau=+1 / tau=-1 add/sub ops
# (X[a'] = T[slot] +/- U[slot]).
_STAGE_OPS = {
    6: dict(  # rot' [4..11], pair-xor 12
        t=[dict(xoff=4, xdims=[[1, 8]], toff=0, tdims=[[1, 8]])],
        u=[dict(xoff=8, xdims=[[-4, 2], [1, 4]], toff=0, tdims=[[4, 2], [1, 4]])],
        ap=[dict(xoff=5, xdims=[[3, 2], [2, 2]], toff=1, tdims=[[3, 2], [2, 2]])],
        am=[dict(xoff=4, xdims=[[5, 2], [2, 2]], toff=0, tdims=[[5, 2], [2, 2]])],
    ),
    9: dict(  # rot' [2,3,6,7,10,11,14,15], pair-xor 13
        t=[dict(xoff=2, xdims=[[4, 4], [1, 2]], toff=0, tdims=[[2, 4], [1, 2]])],
        u=[dict(xoff=15, xdims=[[-4, 4], [-1, 2]], toff=0, tdims=[[2, 4], [1, 2]])],
        ap=[dict(xoff=3, xdims=[[3, 2]], toff=1, tdims=[[1, 2]]),
            dict(xoff=10, xdims=[[5, 2]], toff=4, tdims=[[3, 2]])],
        am=[dict(xoff=2, xdims=[[5, 2]], toff=0, tdims=[[3, 2]]),
            dict(xoff=11, xdims=[[3, 2]], toff=5, tdims=[[1, 2]])],
    ),
    5: dict(  # rot' [4,5,6,7,12,13,14,15], pair-xor 11
        t=[dict(xoff=4, xdims=[[8, 2], [1, 4]], toff=0, tdims=[[4, 2], [1, 4]])],
        u=[dict(xoff=15, xdims=[[-8, 2], [-1, 4]], toff=0, tdims=[[4, 2], [1, 4]])],
        ap=[dict(xoff=12, xdims=[[1, 4]], toff=4, tdims=[[1, 4]])],
        am=[dict(xoff=4, xdims=[[1, 4]], toff=0, tdims=[[1, 4]])],
    ),
    3: dict(  # rot' [8..15], pair-xor 7; A ops split by comp halves so the
        # upper-half output DMA can fire while the lower half finishes
        t=[dict(xoff=8, xdims=[[1, 8]], toff=0, tdims=[[1, 8]])],
        u=[dict(xoff=15, xdims=[[-1, 8]], toff=0, tdims=[[1, 8]])],
        ap=[dict(xoff=9, xdims=[[2, 2]], toff=1, tdims=[[2, 2]]),
            dict(xoff=13, xdims=[[2, 2]], toff=5, tdims=[[2, 2]])],
        am=[dict(xoff=8, xdims=[[2, 2]], toff=0, tdims=[[2, 2]]),
            dict(xoff=12, xdims=[[2, 2]], toff=4, tdims=[[2, 2]])],
    ),
}


def _iter_idx(dims):
    import itertools
    return itertools.product(*[range(c) for (_, c) in dims])


def _expand(sub, off_key, dims_key):
    """Yield (linear_index, multi_index) pairs for a descriptor sub-op."""
    for idx in _iter_idx(sub[dims_key]):
        yield sub[off_key] + sum(s * i for (s, _), i in zip(sub[dims_key], idx))


def _verify_stage_ops(cayley):
    """Symbolically apply the descriptor index arithmetic for one position
    and check it matches the Cayley-derived Givens stage for every plane.
    Descriptors are in SIGMA-permuted component coordinates."""
    for m in STAGE_ORDER:
        ops = _STAGE_OPS[m]
        q = SIGMA[m]
        rot = sorted(SIGMA[a] for a in range(MV)
                     if bin(a & m).count("1") == 1)
        slot_of = {a: s for s, a in enumerate(rot)}
        tmap, umap = {}, {}
        for sub in ops["t"]:
            for slot, comp in zip(_expand(sub, "toff", "tdims"),
                                  _expand(sub, "xoff", "xdims")):
                tmap[slot] = comp
        for sub in ops["u"]:
            for slot, comp in zip(_expand(sub, "toff", "tdims"),
                                  _expand(sub, "xoff", "xdims")):
                umap[slot] = comp
        amap = {}
        for key, sgn in (("ap", 1), ("am", -1)):
            for sub in ops[key]:
                for slot, comp in zip(_expand(sub, "toff", "tdims"),
                                      _expand(sub, "xoff", "xdims")):
                    amap[slot] = (comp, sgn)
        assert sorted(tmap) == sorted(umap) == sorted(amap) == list(range(8)), m
        for slot in range(8):
            ap_, sgn = amap[slot]
            assert slot_of[ap_] == slot, (m, slot, ap_)
            assert tmap[slot] == ap_, (m, slot, "cos part must read dst comp")
            bp = ap_ ^ q
            assert umap[slot] == bp, (m, slot, umap[slot], bp)
            a_old = SIGMA[ap_]  # SIGMA is an involution
            tau = float(cayley[a_old, m, a_old ^ m])
            assert sgn == tau, (m, slot, sgn, tau)


def _numeric_check(cayley):
    """End-to-end numpy check of the 4-stage ladder vs the rotor sandwich."""
    rng = np.random.default_rng(0)
    x = rng.standard_normal(MV)
    angles = {m: rng.standard_normal() for m in STAGE_ORDER}

    def gp(A, Bv):
        return np.einsum("i,j,ijk->k", A, Bv, cayley)

    # reference order: rotor = ((R3 R5) R9) R6
    rotor = np.zeros(MV); rotor[0] = 1.0
    for m in PLANE_BLADES:
        pr = np.zeros(MV)
        pr[0] = np.cos(angles[m] / 2); pr[m] = np.sin(angles[m] / 2)
        rotor = gp(rotor, pr)
    rrev = rotor.copy()
    for i in range(MV):
        if bin(i).count("1") == 2:
            rrev[i] = -rrev[i]
    want = gp(gp(rotor, x), rrev)

    got = np.empty(MV)
    for a in range(MV):
        got[SIGMA[a]] = x[a]  # host permute into device coords
    for m in STAGE_ORDER:
        c2, s2 = np.cos(angles[m]), np.sin(angles[m])
        ops = _STAGE_OPS[m]
        T, U = np.zeros(8), np.zeros(8)
        for sub in ops["t"]:
            for slot, comp in zip(_expand(sub, "toff", "tdims"),
                                  _expand(sub, "xoff", "xdims")):
                T[slot] = c2 * got[comp]
        for sub in ops["u"]:
            for slot, comp in zip(_expand(sub, "toff", "tdims"),
                                  _expand(sub, "xoff", "xdims")):
                U[slot] = s2 * got[comp]
        nxt = got.copy()
        for key, sgn in (("ap", 1), ("am", -1)):
            for sub in ops[key]:
                for slot, comp in zip(_expand(sub, "toff", "tdims"),
                                      _expand(sub, "xoff", "xdims")):
                    nxt[comp] = T[slot] + sgn * U[slot]
        got = nxt
    got_ref = np.array([got[SIGMA[a]] for a in range(MV)])
    assert np.allclose(got_ref, want, atol=1e-10), (got_ref, want)


def _ap_with_dims(base_ap, extra_off, dims):
    ap = [list(base_ap.ap[0])] + [list(d) for d in dims]
    return bass.AP(base_ap.tensor, base_ap.offset + extra_off, ap)


def _build_program(g2ps):
    """g2ps[i] = freqs[i]*coefs[i]/(2pi) for plane index i (blade order
    PLANE_BLADES)."""
    nc = bacc.Bacc("TRN2", target_bir_lowering=False, debug=False,
                   enable_asserts=False, num_devices=NCORES)
    # device comps 2..15 only; comps 0/1 (scalar+pseudoscalar) pass through
    # on the host
    xh_d = nc.dram_tensor("xh", [P, 14 * J], F16, kind="ExternalInput")
    pos_d = nc.dram_tensor("pos", [P, J], I16, kind="ExternalInput")
    out_d = nc.dram_tensor("out", [P, 14 * J], F16, kind="ExternalOutput")

    SIN = mybir.ActivationFunctionType.Sin
    # positive shift so fmod == python mod; power of two well above max |q|
    maxq = max(abs(g) for g in g2ps) * (MAX_LEN - 1)
    shift = float(2.0 ** int(np.ceil(np.log2(maxq + 2.0))))

    def stage_ap(base, sub, off_key, dims_key, jh=None):
        off = sub[off_key] * J
        dims = [[s * J, n] for s, n in sub[dims_key]]
        if jh is None:
            return _ap_with_dims(base, off, dims + [[1, J]])
        return _ap_with_dims(base, off + jh * (J // 2), dims + [[1, J // 2]])

    def bcast_ap(base, i, sub, dims_key, jh=None):
        dims = [[0, n] for _, n in sub[dims_key]]
        if jh is None:
            return _ap_with_dims(base, i * J, dims + [[1, J]])
        return _ap_with_dims(base, i * J + jh * (J // 2), dims + [[1, J // 2]])

    with tile.TileContext(nc) as tc:
        with tc.tile_pool(name="ang", bufs=1) as apool, \
             tc.tile_pool(name="x", bufs=1) as xpool, \
             tc.tile_pool(name="tmp", bufs=2) as tpool:

            # ---- DMAs up front.  pos halves go down both hwdge queues in
            # parallel (it gates the whole angle chain); x arrives in two
            # pieces with the first stage's comps (4..11) first ----
            Pp = apool.tile([P, J], I16)
            nc.sync.dma_start(Pp[:], pos_d[:])
            X = xpool.tile([P, MV * J], F16)
            # first stage's comps get the scalar hwdge queue to themselves
            # (every DMA has a ~2.4us fixed latency, so queue order is what
            # determines arrival); later stages' comps follow pos on sync
            nc.scalar.dma_start(X[:, 4 * J:12 * J], xh_d[:, 2 * J:10 * J])
            nc.sync.dma_start(X[:, 2 * J:4 * J], xh_d[:, :2 * J])
            nc.sync.dma_start(X[:, 12 * J:], xh_d[:, 10 * J:])

            posf = apool.tile([P, J], F32)
            nc.vector.tensor_copy(posf[:], Pp[:])

            QP = apool.tile([P, 4 * J], F32)
            Kr = apool.tile([P, 4 * J], F32)
            FR = apool.tile([P, 4 * J], F32)
            AB = apool.tile([P, 4 * J], F32)
            C2 = apool.tile([P, 4 * J], F16)
            S2 = apool.tile([P, 4 * J], F16)
            HB = apool.tile([P, 1], F32)
            nc.vector.memset(HB[:], HALF_PI)
            MB = apool.tile([P, 1], F32)
            nc.vector.memset(MB[:], MAGIC)
            NMB = apool.tile([P, 1], F32)
            nc.vector.memset(NMB[:], -MAGIC)

            IDN = mybir.ActivationFunctionType.Identity
            ABSF = mybir.ActivationFunctionType.Abs

            def sin_acts(i):
                # c2 = cos(2pi*FR) = sin(pi/2 - 2pi*|FR|) ; s2 = sin(2pi*FR)
                # c2 first: the ladder's T op (which only needs c2) leads
                sl = slice(i * J, (i + 1) * J)
                nc.scalar.activation(C2[:, sl], AB[:, sl], SIN,
                                     bias=HB[:], scale=-TWO_PI)
                nc.scalar.activation(S2[:, sl], FR[:, sl], SIN, scale=TWO_PI)

            def angle_chain_vec(i):
                # FR = q - round(q) in [-1/2,1/2] => angle A == 2pi*FR
                sl = slice(i * J, (i + 1) * J)
                nc.vector.tensor_scalar_mul(QP[:, sl], posf[:], float(g2ps[i]))
                nc.vector.tensor_scalar(Kr[:, sl], QP[:, sl], MAGIC, MAGIC,
                                        ALU.add, ALU.subtract)
                nc.vector.tensor_sub(FR[:, sl], QP[:, sl], Kr[:, sl])
                # |FR| by clearing the fp32 sign bit (abs_max is not a
                # valid tensor_scalar ALU op on HW)
                nc.vector.tensor_scalar(
                    AB[:, sl].bitcast(mybir.dt.uint32),
                    FR[:, sl].bitcast(mybir.dt.uint32),
                    0x7FFFFFFF, None, ALU.bitwise_and)
                sin_acts(i)

            def angle_chain_aux(i):
                # same chain, but off the vector engine: ScalarE does the
                # scalar pieces (round(q) via the magic constant as two
                # Identity-with-bias activations), gpsimd the tensor-tensor
                # subtract.  Runs concurrently with the ladder's stages.
                sl = slice(i * J, (i + 1) * J)
                nc.scalar.mul(QP[:, sl], posf[:], float(g2ps[i]))
                nc.scalar.activation(Kr[:, sl], QP[:, sl], IDN, bias=MB[:])
                nc.scalar.activation(Kr[:, sl], Kr[:, sl], IDN, bias=NMB[:])
                nc.gpsimd.tensor_sub(FR[:, sl], QP[:, sl], Kr[:, sl])
                nc.scalar.activation(AB[:, sl], FR[:, sl], ABSF)
                sin_acts(i)

            # first stage's plane (index 3) on the vector engine for the
            # shortest path to the first tables; the other three planes'
            # chains run on the otherwise-idle scalar+gpsimd engines,
            # overlapping the ladder's early stages on the vector engine
            angle_chain_vec(3)
            for i in (2, 1, 0):
                angle_chain_aux(i)

            # ---- 4 in-place Givens stages on the single x tile ----
            def add_sub(m, key, k):
                sub = _STAGE_OPS[m][key][k]
                fn = nc.vector.tensor_add if key == "ap" else nc.vector.tensor_sub
                fn(stage_ap(X[:], sub, "xoff", "xdims"),
                   stage_ap(T[:], sub, "toff", "tdims"),
                   stage_ap(U[:], sub, "toff", "tdims"))

            for m in STAGE_ORDER:
                i = PLANE_BLADES.index(m)
                ops = _STAGE_OPS[m]
                T = tpool.tile([P, 8 * J], F16, tag="t")
                U = tpool.tile([P, 8 * J], F16, tag="u")

                for sub in ops["t"]:
                    nc.vector.tensor_mul(stage_ap(T[:], sub, "toff", "tdims"),
                                         stage_ap(X[:], sub, "xoff", "xdims"),
                                         bcast_ap(C2[:], i, sub, "tdims"))
                for sub in ops["u"]:
                    nc.vector.tensor_mul(stage_ap(U[:], sub, "toff", "tdims"),
                                         stage_ap(X[:], sub, "xoff", "xdims"),
                                         bcast_ap(S2[:], i, sub, "tdims"))
                if m == 5:
                    # comps 4-7 final after am; comps 2,3 final since m9 ->
                    # lower-half output DMA overlaps the rest of the ladder
                    add_sub(5, "am", 0)
                    nc.sync.dma_start(out_d[:, :6 * J], X[:, 2 * J:8 * J])
                    add_sub(5, "ap", 0)
                elif m == 3:
                    # split by comp halves: DMA comps 8-11 while 12-15 finish
                    add_sub(3, "am", 0)
                    add_sub(3, "ap", 0)
                    nc.sync.dma_start(out_d[:, 6 * J:10 * J], X[:, 8 * J:12 * J])
                    add_sub(3, "am", 1)
                    add_sub(3, "ap", 1)
                    # last piece goes down the idle ScalarE hwdge queue so
                    # its issue overlaps the sync queue's previous DMA
                    nc.scalar.dma_start(out_d[:, 10 * J:], X[:, 12 * J:])
                else:
                    for key in ("ap", "am"):
                        for k in range(len(ops[key])):
                            add_sub(m, key, k)

    nc.compile()
    return nc


_PROGRAM_CACHE = {}


def _get_program(g2ps):
    key = tuple(g2ps)
    if key not in _PROGRAM_CACHE:
        _PROGRAM_CACHE[key] = _build_program(g2ps)
    return _PROGRAM_CACHE[key]


def _derive_g2ps(theta, bx, by, bz, bw):
    coefs = [float(np.asarray(c, dtype=np.float32).reshape(MV)[b])
             for c, b in zip((bx, by, bz, bw), PLANE_BLADES)]
    theta = np.asarray(theta, dtype=np.float32)
    freqs = [float(theta.reshape(MAX_LEN, 4)[1, i]) for i in range(4)]
    th_check = np.arange(MAX_LEN, dtype=np.float32)[:, None] * \
        np.asarray(freqs, dtype=np.float32)[None, :]
    assert np.array_equal(th_check, theta.reshape(MAX_LEN, 4)), \
        "theta table is not linear in position; kernel assumption violated"
    g2ps = [float(np.float64(f) * np.float64(c) / (2.0 * np.pi))
            for f, c in zip(freqs, coefs)]
    # magic rounding needs |q| < 2^22
    assert max(abs(g) for g in g2ps) * (MAX_LEN - 1) < 2 ** 22
    return g2ps


# reference comp index for each device column block 2..15 (SIGMA inverse of
# the device comp id; SIGMA is an involution)
_DEV_COMPS = [SIGMA[c] for c in range(2, MV)]


def prep_in_maps(x, pos):
    """Host marshaling: per-core SIGMA-permuted comp-planar fp16 x tiles
    (device comps 2..15 only) + int16 pos."""
    x = np.asarray(x, dtype=np.float32)
    pos_i = np.clip(np.asarray(pos), 0, MAX_LEN - 1).astype(np.int16)
    in_maps = []
    for g in range(NCORES):
        rows = np.ascontiguousarray(x[g * ROWS_PER_CORE:(g + 1) * ROWS_PER_CORE])
        # [P, J, MV] -> select ref comps for device blocks -> [P, 14, J]
        xr = rows.reshape(P, J, MV)[:, :, _DEV_COMPS].transpose(0, 2, 1)
        xg = np.ascontiguousarray(xr).reshape(P, 14 * J).astype(np.float16)
        pg = np.ascontiguousarray(
            pos_i[g * ROWS_PER_CORE:(g + 1) * ROWS_PER_CORE]).reshape(P, J)
        in_maps.append({"xh": xg, "pos": pg})
    return in_maps


def unshard_out(core_out, rows_x):
    """[P, 14*J] fp16 comp-planar (device comps 2..15) -> (R, L, MV) fp32.
    Device comps 0/1 = reference comps 0/15 pass through from the input."""
    o = np.asarray(core_out).reshape(P, 14, J).transpose(0, 2, 1)
    full = np.empty((P, J, MV), dtype=np.float32)
    full[:, :, _DEV_COMPS] = o.astype(np.float32)
    full[:, :, 0] = rows_x[:, :, 0]
    full[:, :, 15] = rows_x[:, :, 15]
    return full.reshape(ROWS_PER_CORE, L, MV)


def kernel(x, pos, bx, by, bz, bw, theta, cayley, biv_mask, scalar_mask):
    x = np.asarray(x, dtype=np.float32)
    pos = np.asarray(pos)
    cayley = np.asarray(cayley, dtype=np.float32)

    assert x.shape == (B, L, MV) and pos.shape == (B, L)

    _verify_stage_ops(cayley)
    _numeric_check(cayley)

    g2ps = _derive_g2ps(theta, bx, by, bz, bw)
    nc = _get_program(g2ps)

    in_maps = prep_in_maps(x, pos)
    res = run_bass_kernel_spmd(nc, in_maps, core_ids=list(range(NCORES)))
    out = np.empty((B, L, MV), dtype=np.float32)
    for g in range(NCORES):
        rows_x = x[g * ROWS_PER_CORE:(g + 1) * ROWS_PER_CORE].reshape(P, J, MV)
        out[g * ROWS_PER_CORE:(g + 1) * ROWS_PER_CORE] = \
            unshard_out(res.results[g]["out"], rows_x)
    return out


# revision 35
# speedup vs baseline: 1.0741x; 1.0741x over previous
"""Trainium2 Bass kernel for CARE position encoding (rotor sandwich product).

The reference computes out = R x R~ where R is a product of 4 plane rotors
(cos(phi_i) + sin(phi_i) e_mi) with phi_i = 0.5 * c_i * theta[pos, i].
Algebraically this factorizes into 4 sequential Givens-rotation stages: for
plane bivector e_m, the 8 basis blades A with |A & m| == 1 rotate in 4
disjoint pairs (A, A^m) by angle 2*phi with pair signs tau = C[A, m, A^m];
the other 8 blades pass through unchanged:
    out[a] = c2*x[a] + tau*s2*x[b] ;  out[b] = c2*x[b] - tau*s2*x[a]

Implementation (data-parallel across 8 cores, batch-sharded, 2 rows/core):
 - component-planar fp16 layout: each x tile is [128, 16 comp-blocks * JT]
   so every DVE tensor_tensor operand has a packed stride-1 innermost dim
   of JT fp16 elements -> the DVE runs them in 2x_1P mode (2 elem/cycle)
   instead of the 1x fp32 mode.
 - host marshals x into this layout (and fp16) per core; the device ladder
   runs four in-place Givens stages (mul cos, mul sin, add/sub split by the
   Cayley pair sign tau); output DMA'd back in fp16 and unpacked on host.
 - angles once per core: q_i = float(pos) * (f_i*c_i/2pi); round via the
   1.5*2^23 magic constant; FR = q - round(q) in [-1/2,1/2] so the rotation
   angle A_i == 2pi*FR (mod 2pi).  ScalarE Sin evaluates both tables with
   the affine pre-scale folded into the activation: s2 = Sin(2pi*FR),
   c2 = Sin(-2pi*|FR| + pi/2) = cos(A).
 - every stage's index arithmetic is verified symbolically against the
   input Cayley tensor at kernel() time.
"""
import numpy as np

import concourse.bass as bass
import concourse.tile as tile
from concourse import bacc, mybir
from concourse.bass_utils import run_bass_kernel_spmd

F32 = mybir.dt.float32
F16 = mybir.dt.float16
I16 = mybir.dt.int16
I32 = mybir.dt.int32
ALU = mybir.AluOpType

P = 128
NCORES = 8
B, L, MV = 16, 16384, 16
MAX_LEN = 16384
ROWS_PER_CORE = B // NCORES          # 2
N = ROWS_PER_CORE * L                # 32768 positions per core
J = N // P                           # 256 positions per partition
JT = J                               # single x-tile covering all positions
NT = J // JT

PLANE_BLADES = (3, 5, 9, 6)          # reference plane order (rotor build)
STAGE_ORDER = (6, 9, 5, 3)           # sandwich stage application order

MAGIC = float(np.float32(1.5 * 2 ** 23))
TWO_PI = float(2.0 * np.pi)
HALF_PI = float(np.pi / 2.0)


def _register_frac_op():
    """Fused custom DVE op: out = in0 - ((in0 + C0) - C0), i.e. the
    round-via-magic-constant fractional part in one instruction.  The uop
    sha is computed at runtime so the pin always matches this install."""
    from concourse import dve_ops as D
    from concourse.dve_spec import Spec, Src0, C0, lower, _has_src1
    from concourse.dve_uop import DveOpSpec

    name = "FRAC_MAGIC_ANT"
    if name in D._SUB_OPCODE_FOR_NAME:
        return next(o for o in D.OPS if o.name == name)
    def _ref(in0, in1, s0, s1, imm2):
        f32 = np.float32
        return f32(in0) - (f32(f32(in0) + f32(s0)) - f32(s0))

    spec = Spec(body=Src0 - ((Src0 + C0) - C0), reference=_ref)
    row = max(D._SUB_OPCODE_FOR_NAME.values()) + 1
    assert row < 0x20, "custom DVE opcode rows exhausted"
    D._SUB_OPCODE_FOR_NAME[name] = row
    shas = {}
    for ver in ("v3", "v4"):
        uops = lower(spec, ver=ver)
        shas[ver] = DveOpSpec(name=name, opcode=row, uops=uops,
                              rd1_en=_has_src1(spec)).sha(ver)
    op = D.DveOp(name, spec, subdim=False, uops_sha=shas)
    D.OPS.append(op)
    D.CUSTOM_DVE_SPECS[name] = spec
    return op

# Component permutation: device comp a' = SIGMA[a] for reference comp a.
# GF(2)-linear change of basis chosen so every stage's rotating set is a
# contiguous / single-stride block: b0'=bit0, b1'=bit0^bit3, b2'=bit0^bit2,
# b3'=bit0^bit1.  SIGMA is an involution (SIGMA == its inverse).  Comps 0/1
# (scalar + pseudoscalar) commute with every rotor and pass through
# untouched; the host copies them directly and the device skips them.
SIGMA = (0, 15, 8, 7, 4, 11, 12, 3, 2, 13, 10, 5, 6, 9, 14, 1)

# Per-stage op descriptors in permuted component-planar layout.  comp dims
# are in units of J columns; slot = rank of the destination comp a' within
# the sorted rotating set.  t/u read X at xoff+xdims, write T/U at
# toff+tdims; ap/am are the tau=+1 / tau=-1 add/sub ops
# (X[a'] = T[slot] +/- U[slot]).
_STAGE_OPS = {
    6: dict(  # rot' [4..11], pair-xor 12
        t=[dict(xoff=4, xdims=[[1, 8]], toff=0, tdims=[[1, 8]])],
        u=[dict(xoff=8, xdims=[[-4, 2], [1, 4]], toff=0, tdims=[[4, 2], [1, 4]])],
        ap=[dict(xoff=5, xdims=[[3, 2], [2, 2]], toff=1, tdims=[[3, 2], [2, 2]])],
        am=[dict(xoff=4, xdims=[[5, 2], [2, 2]], toff=0, tdims=[[5, 2], [2, 2]])],
    ),
    9: dict(  # rot' [2,3,6,7,10,11,14,15], pair-xor 13
        t=[dict(xoff=2, xdims=[[4, 4], [1, 2]], toff=0, tdims=[[2, 4], [1, 2]])],
        u=[dict(xoff=15, xdims=[[-4, 4], [-1, 2]], toff=0, tdims=[[2, 4], [1, 2]])],
        ap=[dict(xoff=3, xdims=[[3, 2]], toff=1, tdims=[[1, 2]]),
            dict(xoff=10, xdims=[[5, 2]], toff=4, tdims=[[3, 2]])],
        am=[dict(xoff=2, xdims=[[5, 2]], toff=0, tdims=[[3, 2]]),
            dict(xoff=11, xdims=[[3, 2]], toff=5, tdims=[[1, 2]])],
    ),
    5: dict(  # rot' [4,5,6,7,12,13,14,15], pair-xor 11
        t=[dict(xoff=4, xdims=[[8, 2], [1, 4]], toff=0, tdims=[[4, 2], [1, 4]])],
        u=[dict(xoff=15, xdims=[[-8, 2], [-1, 4]], toff=0, tdims=[[4, 2], [1, 4]])],
        ap=[dict(xoff=12, xdims=[[1, 4]], toff=4, tdims=[[1, 4]])],
        am=[dict(xoff=4, xdims=[[1, 4]], toff=0, tdims=[[1, 4]])],
    ),
    3: dict(  # rot' [8..15], pair-xor 7; A ops split by comp halves so the
        # upper-half output DMA can fire while the lower half finishes
        t=[dict(xoff=8, xdims=[[1, 8]], toff=0, tdims=[[1, 8]])],
        u=[dict(xoff=15, xdims=[[-1, 8]], toff=0, tdims=[[1, 8]])],
        ap=[dict(xoff=9, xdims=[[2, 2]], toff=1, tdims=[[2, 2]]),
            dict(xoff=13, xdims=[[2, 2]], toff=5, tdims=[[2, 2]])],
        am=[dict(xoff=8, xdims=[[2, 2]], toff=0, tdims=[[2, 2]]),
            dict(xoff=12, xdims=[[2, 2]], toff=4, tdims=[[2, 2]])],
    ),
}


def _iter_idx(dims):
    import itertools
    return itertools.product(*[range(c) for (_, c) in dims])


def _expand(sub, off_key, dims_key):
    """Yield (linear_index, multi_index) pairs for a descriptor sub-op."""
    for idx in _iter_idx(sub[dims_key]):
        yield sub[off_key] + sum(s * i for (s, _), i in zip(sub[dims_key], idx))


def _verify_stage_ops(cayley):
    """Symbolically apply the descriptor index arithmetic for one position
    and check it matches the Cayley-derived Givens stage for every plane.
    Descriptors are in SIGMA-permuted component coordinates."""
    for m in STAGE_ORDER:
        ops = _STAGE_OPS[m]
        q = SIGMA[m]
        rot = sorted(SIGMA[a] for a in range(MV)
                     if bin(a & m).count("1") == 1)
        slot_of = {a: s for s, a in enumerate(rot)}
        tmap, umap = {}, {}
        for sub in ops["t"]:
            for slot, comp in zip(_expand(sub, "toff", "tdims"),
                                  _expand(sub, "xoff", "xdims")):
                tmap[slot] = comp
        for sub in ops["u"]:
            for slot, comp in zip(_expand(sub, "toff", "tdims"),
                                  _expand(sub, "xoff", "xdims")):
                umap[slot] = comp
        amap = {}
        for key, sgn in (("ap", 1), ("am", -1)):
            for sub in ops[key]:
                for slot, comp in zip(_expand(sub, "toff", "tdims"),
                                      _expand(sub, "xoff", "xdims")):
                    amap[slot] = (comp, sgn)
        assert sorted(tmap) == sorted(umap) == sorted(amap) == list(range(8)), m
        for slot in range(8):
            ap_, sgn = amap[slot]
            assert slot_of[ap_] == slot, (m, slot, ap_)
            assert tmap[slot] == ap_, (m, slot, "cos part must read dst comp")
            bp = ap_ ^ q
            assert umap[slot] == bp, (m, slot, umap[slot], bp)
            a_old = SIGMA[ap_]  # SIGMA is an involution
            tau = float(cayley[a_old, m, a_old ^ m])
            assert sgn == tau, (m, slot, sgn, tau)


def _numeric_check(cayley):
    """End-to-end numpy check of the 4-stage ladder vs the rotor sandwich."""
    rng = np.random.default_rng(0)
    x = rng.standard_normal(MV)
    angles = {m: rng.standard_normal() for m in STAGE_ORDER}

    def gp(A, Bv):
        return np.einsum("i,j,ijk->k", A, Bv, cayley)

    # reference order: rotor = ((R3 R5) R9) R6
    rotor = np.zeros(MV); rotor[0] = 1.0
    for m in PLANE_BLADES:
        pr = np.zeros(MV)
        pr[0] = np.cos(angles[m] / 2); pr[m] = np.sin(angles[m] / 2)
        rotor = gp(rotor, pr)
    rrev = rotor.copy()
    for i in range(MV):
        if bin(i).count("1") == 2:
            rrev[i] = -rrev[i]
    want = gp(gp(rotor, x), rrev)

    got = np.empty(MV)
    for a in range(MV):
        got[SIGMA[a]] = x[a]  # host permute into device coords
    for m in STAGE_ORDER:
        c2, s2 = np.cos(angles[m]), np.sin(angles[m])
        ops = _STAGE_OPS[m]
        T, U = np.zeros(8), np.zeros(8)
        for sub in ops["t"]:
            for slot, comp in zip(_expand(sub, "toff", "tdims"),
                                  _expand(sub, "xoff", "xdims")):
                T[slot] = c2 * got[comp]
        for sub in ops["u"]:
            for slot, comp in zip(_expand(sub, "toff", "tdims"),
                                  _expand(sub, "xoff", "xdims")):
                U[slot] = s2 * got[comp]
        nxt = got.copy()
        for key, sgn in (("ap", 1), ("am", -1)):
            for sub in ops[key]:
                for slot, comp in zip(_expand(sub, "toff", "tdims"),
                                      _expand(sub, "xoff", "xdims")):
                    nxt[comp] = T[slot] + sgn * U[slot]
        got = nxt
    got_ref = np.array([got[SIGMA[a]] for a in range(MV)])
    assert np.allclose(got_ref, want, atol=1e-10), (got_ref, want)


def _ap_with_dims(base_ap, extra_off, dims):
    ap = [list(base_ap.ap[0])] + [list(d) for d in dims]
    return bass.AP(base_ap.tensor, base_ap.offset + extra_off, ap)


def _build_program(g2ps):
    """g2ps[i] = freqs[i]*coefs[i]/(2pi) for plane index i (blade order
    PLANE_BLADES)."""
    nc = bacc.Bacc("TRN2", target_bir_lowering=False, debug=False,
                   enable_asserts=False, num_devices=NCORES)
    # device comps 2..15 only; comps 0/1 (scalar+pseudoscalar) pass through
    # on the host
    xh_d = nc.dram_tensor("xh", [P, 14 * J], F16, kind="ExternalInput")
    pos_d = nc.dram_tensor("pos", [P, J], I16, kind="ExternalInput")
    out_d = nc.dram_tensor("out", [P, 14 * J], F16, kind="ExternalOutput")

    SIN = mybir.ActivationFunctionType.Sin

    def stage_ap(base, sub, off_key, dims_key, jh=None):
        off = sub[off_key] * J
        dims = [[s * J, n] for s, n in sub[dims_key]]
        if jh is None:
            return _ap_with_dims(base, off, dims + [[1, J]])
        return _ap_with_dims(base, off + jh * (J // 2), dims + [[1, J // 2]])

    def bcast_ap(base, i, sub, dims_key, jh=None):
        dims = [[0, n] for _, n in sub[dims_key]]
        if jh is None:
            return _ap_with_dims(base, i * J, dims + [[1, J]])
        return _ap_with_dims(base, i * J + jh * (J // 2), dims + [[1, J // 2]])

    with tile.TileContext(nc) as tc:
        with tc.tile_pool(name="ang", bufs=1) as apool, \
             tc.tile_pool(name="x", bufs=1) as xpool, \
             tc.tile_pool(name="tmp", bufs=2) as tpool:

            # ---- DMAs up front.  pos halves go down both hwdge queues in
            # parallel (it gates the whole angle chain); x arrives in two
            # pieces with the first stage's comps (4..11) first ----
            # pos gates the whole angle chain and every DMA has a ~2.4us
            # fixed latency, so its halves go down both hwdge queues in
            # parallel; the x pieces follow (first stage's comps first)
            Pp = apool.tile([P, J], I16)
            nc.sync.dma_start(Pp[:P // 2], pos_d[:P // 2])
            nc.scalar.dma_start(Pp[P // 2:], pos_d[P // 2:])
            X = xpool.tile([P, MV * J], F16)
            nc.scalar.dma_start(X[:, 4 * J:12 * J], xh_d[:, 2 * J:10 * J])
            nc.sync.dma_start(X[:, 2 * J:4 * J], xh_d[:, :2 * J])
            nc.sync.dma_start(X[:, 12 * J:], xh_d[:, 10 * J:])

            posf = apool.tile([P, J], F32)
            nc.vector.tensor_copy(posf[:], Pp[:])

            QP = apool.tile([P, 4 * J], F32)
            Kr = apool.tile([P, 4 * J], F32)
            FR = apool.tile([P, 4 * J], F32)
            AB = apool.tile([P, 4 * J], F32)
            C2 = apool.tile([P, 4 * J], F16)
            S2 = apool.tile([P, 4 * J], F16)
            HB = apool.tile([P, 1], F32)
            nc.vector.memset(HB[:], HALF_PI)

            frac_op = _register_frac_op()

            def angle_chain(sl, g2p_slice):
                # FR = q - round(q) in [-1/2,1/2] => angle A == 2pi*FR
                for k, g in enumerate(g2p_slice):
                    ssl = slice(sl.start + k * J, sl.start + (k + 1) * J)
                    nc.vector.tensor_scalar_mul(QP[:, ssl], posf[:], float(g))
                nc.vector._custom_dve(frac_op, out=FR[:, sl], in0=QP[:, sl],
                                      s0=MAGIC)
                # |FR| by clearing the fp32 sign bit (abs_max is not a
                # valid tensor_scalar ALU op on HW)
                nc.vector.tensor_scalar(
                    AB[:, sl].bitcast(mybir.dt.uint32),
                    FR[:, sl].bitcast(mybir.dt.uint32),
                    0x7FFFFFFF, None, ALU.bitwise_and)
                # c2 = cos(2pi*FR) = sin(pi/2 - 2pi*|FR|) ; s2 = sin(2pi*FR)
                # c2 first: the ladder's T op (which only needs c2) leads
                nc.scalar.activation(C2[:, sl], AB[:, sl], SIN,
                                     bias=HB[:], scale=-TWO_PI)
                nc.scalar.activation(S2[:, sl], FR[:, sl], SIN, scale=TWO_PI)

            # first stage's plane (index 3) first so its tables are ready
            # early; the other three (indices 0..2, contiguous) batched.
            # (Offloading these chains to ScalarE/GpSimd was tried and lost:
            # ScalarE ACT latency misses the stage deadlines, and concurrent
            # GpSimd SBUF traffic slows DVE tensor_tensor ops by ~40%.)
            angle_chain(slice(3 * J, 4 * J), [g2ps[3]])
            angle_chain(slice(0, 3 * J), g2ps[0:3])

            # ---- 4 in-place Givens stages on the single x tile ----
            def add_sub(m, key, k):
                sub = _STAGE_OPS[m][key][k]
                fn = nc.vector.tensor_add if key == "ap" else nc.vector.tensor_sub
                fn(stage_ap(X[:], sub, "xoff", "xdims"),
                   stage_ap(T[:], sub, "toff", "tdims"),
                   stage_ap(U[:], sub, "toff", "tdims"))

            for m in STAGE_ORDER:
                i = PLANE_BLADES.index(m)
                ops = _STAGE_OPS[m]
                T = tpool.tile([P, 8 * J], F16, tag="t")
                U = tpool.tile([P, 8 * J], F16, tag="u")

                for sub in ops["t"]:
                    nc.vector.tensor_mul(stage_ap(T[:], sub, "toff", "tdims"),
                                         stage_ap(X[:], sub, "xoff", "xdims"),
                                         bcast_ap(C2[:], i, sub, "tdims"))
                for sub in ops["u"]:
                    nc.vector.tensor_mul(stage_ap(U[:], sub, "toff", "tdims"),
                                         stage_ap(X[:], sub, "xoff", "xdims"),
                                         bcast_ap(S2[:], i, sub, "tdims"))
                if m == 5:
                    # comps 4-7 final after am; comps 2,3 final since m9 ->
                    # lower-half output DMA overlaps the rest of the ladder
                    add_sub(5, "am", 0)
                    nc.sync.dma_start(out_d[:, :6 * J], X[:, 2 * J:8 * J])
                    add_sub(5, "ap", 0)
                elif m == 3:
                    # split by comp halves: DMA comps 8-11 while 12-15 finish
                    add_sub(3, "am", 0)
                    add_sub(3, "ap", 0)
                    nc.sync.dma_start(out_d[:, 6 * J:10 * J], X[:, 8 * J:12 * J])
                    add_sub(3, "am", 1)
                    add_sub(3, "ap", 1)
                    # last piece goes down the idle ScalarE hwdge queue so
                    # its issue overlaps the sync queue's previous DMA
                    nc.scalar.dma_start(out_d[:, 10 * J:], X[:, 12 * J:])
                else:
                    for key in ("ap", "am"):
                        for k in range(len(ops[key])):
                            add_sub(m, key, k)

    nc.compile()
    return nc


_PROGRAM_CACHE = {}


def _get_program(g2ps):
    key = tuple(g2ps)
    if key not in _PROGRAM_CACHE:
        _PROGRAM_CACHE[key] = _build_program(g2ps)
    return _PROGRAM_CACHE[key]


def _derive_g2ps(theta, bx, by, bz, bw):
    coefs = [float(np.asarray(c, dtype=np.float32).reshape(MV)[b])
             for c, b in zip((bx, by, bz, bw), PLANE_BLADES)]
    theta = np.asarray(theta, dtype=np.float32)
    freqs = [float(theta.reshape(MAX_LEN, 4)[1, i]) for i in range(4)]
    th_check = np.arange(MAX_LEN, dtype=np.float32)[:, None] * \
        np.asarray(freqs, dtype=np.float32)[None, :]
    assert np.array_equal(th_check, theta.reshape(MAX_LEN, 4)), \
        "theta table is not linear in position; kernel assumption violated"
    g2ps = [float(np.float64(f) * np.float64(c) / (2.0 * np.pi))
            for f, c in zip(freqs, coefs)]
    # magic rounding needs |q| < 2^22
    assert max(abs(g) for g in g2ps) * (MAX_LEN - 1) < 2 ** 22
    return g2ps


# reference comp index for each device column block 2..15 (SIGMA inverse of
# the device comp id; SIGMA is an involution)
_DEV_COMPS = [SIGMA[c] for c in range(2, MV)]


def prep_in_maps(x, pos):
    """Host marshaling: per-core SIGMA-permuted comp-planar fp16 x tiles
    (device comps 2..15 only) + int16 pos."""
    x = np.asarray(x, dtype=np.float32)
    pos_i = np.clip(np.asarray(pos), 0, MAX_LEN - 1).astype(np.int16)
    in_maps = []
    for g in range(NCORES):
        rows = np.ascontiguousarray(x[g * ROWS_PER_CORE:(g + 1) * ROWS_PER_CORE])
        # [P, J, MV] -> select ref comps for device blocks -> [P, 14, J]
        xr = rows.reshape(P, J, MV)[:, :, _DEV_COMPS].transpose(0, 2, 1)
        xg = np.ascontiguousarray(xr).reshape(P, 14 * J).astype(np.float16)
        pg = np.ascontiguousarray(
            pos_i[g * ROWS_PER_CORE:(g + 1) * ROWS_PER_CORE]).reshape(P, J)
        in_maps.append({"xh": xg, "pos": pg})
    return in_maps


def unshard_out(core_out, rows_x):
    """[P, 14*J] fp16 comp-planar (device comps 2..15) -> (R, L, MV) fp32.
    Device comps 0/1 = reference comps 0/15 pass through from the input."""
    o = np.asarray(core_out).reshape(P, 14, J).transpose(0, 2, 1)
    full = np.empty((P, J, MV), dtype=np.float32)
    full[:, :, _DEV_COMPS] = o.astype(np.float32)
    full[:, :, 0] = rows_x[:, :, 0]
    full[:, :, 15] = rows_x[:, :, 15]
    return full.reshape(ROWS_PER_CORE, L, MV)


def kernel(x, pos, bx, by, bz, bw, theta, cayley, biv_mask, scalar_mask):
    x = np.asarray(x, dtype=np.float32)
    pos = np.asarray(pos)
    cayley = np.asarray(cayley, dtype=np.float32)

    assert x.shape == (B, L, MV) and pos.shape == (B, L)

    _verify_stage_ops(cayley)
    _numeric_check(cayley)

    g2ps = _derive_g2ps(theta, bx, by, bz, bw)
    nc = _get_program(g2ps)

    in_maps = prep_in_maps(x, pos)
    res = run_bass_kernel_spmd(nc, in_maps, core_ids=list(range(NCORES)))
    out = np.empty((B, L, MV), dtype=np.float32)
    for g in range(NCORES):
        rows_x = x[g * ROWS_PER_CORE:(g + 1) * ROWS_PER_CORE].reshape(P, J, MV)
        out[g * ROWS_PER_CORE:(g + 1) * ROWS_PER_CORE] = \
            unshard_out(res.results[g]["out"], rows_x)
    return out
